# revision 2
# baseline (speedup 1.0000x reference)
"""v4 = v3 (quadratic-matmul t12, Relu-clamped zero-row gathers) with the
PSUM recycle cycle broken: each direction's PSUM is two independent
half-tiles (psA_L/psA_H, psB_L/psB_H, 2 banks each), so PE's refill of
one half overlaps ACT/DVE draining the other. The max-fold pairs
positions (s, s+512) within each half; an extra combine level merges
the halves. See kernel_v3.py for the full algebra/correctness argument
(pairing choice does not affect a max-reduction).
"""

import contextlib

import numpy as np

import concourse.bass as bass
import concourse.tile as tile
from concourse import bacc, mybir
from concourse.bass import IndirectOffsetOnAxis
from concourse.bass_utils import run_bass_kernel_spmd

B, S, H, J = 32, 2048, 1024, 20
NCORES = 8
BPC = B // NCORES
P = BPC * J
ROWS = BPC * S
TOTR = ROWS + 1
K = 2 * BPC + 3
CQ = 160.0
CI = 1.0 / 64.0
UNROLL = 16
MMCHUNK = 512
HS = S // 2   # 1024: columns per psum half-tile
Q = S // 4    # 512

f16 = mybir.dt.float16
f32 = mybir.dt.float32
i32 = mybir.dt.int32
Alu = mybir.AluOpType
Act = mybir.ActivationFunctionType


def build_nc(loop_iters: int | None = None) -> bacc.Bacc:
    nc = bacc.Bacc(
        "TRN2", target_bir_lowering=False, debug=False, num_devices=NCORES
    )
    inpf = nc.dram_tensor("inpf", [TOTR, H], f16, kind="ExternalInput").ap()
    inpb = nc.dram_tensor("inpb", [TOTR, H], f16, kind="ExternalInput").ap()
    maskq = nc.dram_tensor("maskq", [2 * BPC, S], f16, kind="ExternalInput").ap()
    statr = nc.dram_tensor("statr", [3, S], f16, kind="ExternalInput").ap()
    wa = nc.dram_tensor("wa", [K, P], f16, kind="ExternalInput").ap()
    wb = nc.dram_tensor("wb", [K, P], f16, kind="ExternalInput").ap()
    consts = nc.dram_tensor("consts", [P, 2], f32, kind="ExternalInput").ap()
    out = nc.dram_tensor("out", [P, 2 * H], f16, kind="ExternalOutput").ap()

    n_iters = loop_iters if loop_iters is not None else 1
    unroll = UNROLL if loop_iters is not None else 1
    bb = 2 if loop_iters is not None else 1
    rb = 8 if loop_iters is not None else 1

    with tile.TileContext(nc) as tc:
        with contextlib.ExitStack() as stk:
            static = stk.enter_context(tc.tile_pool(name="static", bufs=1))
            psum = stk.enter_context(
                tc.tile_pool(name="psum", bufs=1, space="PSUM")
            )
            consts_sb = static.tile([P, 2], f32)
            wa_sb = static.tile([K, P], f16)
            wb_sb = static.tile([K, P], f16)
            nc.scalar.dma_start(consts_sb[:], consts[:])
            nc.scalar.dma_start(wa_sb[:], wa[:])
            nc.scalar.dma_start(wb_sb[:], wb[:])

            maskring = [
                static.tile([K, S], f16, name=f"mq{i}") for i in range(rb)
            ]
            for t in maskring:
                nc.scalar.dma_start(t[0:3, :], statr[:])
            psAL = psum.tile([P, HS], f32, name="psAL")
            psAH = psum.tile([P, HS], f32, name="psAH")
            psBL = psum.tile([P, HS], f32, name="psBL")
            psBH = psum.tile([P, HS], f32, name="psBH")

            def load(pipe, iv):
                mq = pipe.intermediate_tile(
                    [K, S], f16, name="mq", bufs=rb, prealloc=maskring
                )
                nc.sync.dma_start(mq[3:11, :], maskq[:])
                return mq

            def mm(pipe, iv, mq):
                pA_L = pipe.intermediate_tile(
                    [P, HS], f32, bufs=1, name="pA_L", prealloc=[psAL]
                )
                pA_H = pipe.intermediate_tile(
                    [P, HS], f32, bufs=1, name="pA_H", prealloc=[psAH]
                )
                pB_L = pipe.intermediate_tile(
                    [P, HS], f32, bufs=1, name="pB_L", prealloc=[psBL]
                )
                pB_H = pipe.intermediate_tile(
                    [P, HS], f32, bufs=1, name="pB_H", prealloc=[psBH]
                )
                for dst, lo in ((pA_L, 0), (pA_H, HS)):
                    for n in range(0, HS, MMCHUNK):
                        nc.tensor.matmul(
                            dst[:, n : n + MMCHUNK], wa_sb[:],
                            mq[:, lo + n : lo + n + MMCHUNK],
                        )
                for dst, lo in ((pB_L, 0), (pB_H, HS)):
                    for n in range(0, HS, MMCHUNK):
                        nc.tensor.matmul(
                            dst[:, n : n + MMCHUNK], wb_sb[:],
                            mq[:, lo + n : lo + n + MMCHUNK],
                        )
                return pA_L, pA_H, pB_L, pB_H

            def fold(pipe, iv, ps):
                pA_L, pA_H, pB_L, pB_H = ps
                # ACT drains the low quarter of each A-half and all of B to
                # f16 (DVE may read at most one PSUM operand per op).
                aL = pipe.intermediate_tile([P, Q], f16, bufs=bb, name="aL")
                aH = pipe.intermediate_tile([P, Q], f16, bufs=bb, name="aH")
                b16 = pipe.intermediate_tile([P, S], f16, bufs=bb, name="b16")
                nc.scalar.activation(
                    out=aL[:], in_=pA_L[:, 0:Q], func=Act.Identity, scale=1.0
                )
                nc.scalar.activation(
                    out=b16[:, 0:HS], in_=pB_L[:], func=Act.Identity, scale=1.0
                )
                nc.scalar.activation(
                    out=aH[:], in_=pA_H[:, 0:Q], func=Act.Identity, scale=1.0
                )
                nc.scalar.activation(
                    out=b16[:, HS:S], in_=pB_H[:], func=Act.Identity, scale=1.0
                )
                # DVE folds: A halves pair (s, s+512) in-half vs f16 copy.
                fa = pipe.intermediate_tile([P, S // 2], f16, bufs=1, name="fa")
                nc.vector.tensor_tensor(
                    out=fa[:, 0:Q], in0=aL[:], in1=pA_L[:, Q:HS], op=Alu.max
                )
                nc.vector.tensor_tensor(
                    out=fa[:, Q : 2 * Q], in0=aH[:], in1=pA_H[:, Q:HS],
                    op=Alu.max,
                )
                # t512: dir A combined to 512, dir B folded 2048->512.
                t512 = pipe.intermediate_tile([P, 2 * Q], f16, bufs=1, name="t512")
                fb = pipe.intermediate_tile([P, S // 2], f16, bufs=1, name="fb")
                nc.vector.tensor_tensor(
                    out=t512[:, 0:Q], in0=fa[:, 0:Q], in1=fa[:, Q : 2 * Q],
                    op=Alu.max,
                )
                nc.vector.tensor_tensor(
                    out=fb[:], in0=b16[:, 0:HS], in1=b16[:, HS:S], op=Alu.max
                )
                nc.vector.tensor_tensor(
                    out=t512[:, Q : 2 * Q], in0=fb[:, 0:Q], in1=fb[:, Q : 2 * Q],
                    op=Alu.max,
                )
                # joint tree on [P, 2, 512]
                u = pipe.intermediate_tile([P, Q], f16, bufs=1, name="u")
                v = pipe.intermediate_tile([P, Q // 2], f16, bufs=1, name="v")
                red = pipe.intermediate_tile([P, 2], f16, bufs=rb, name="red")
                tv = t512[:].rearrange("p (k s) -> p k s", k=2)
                uv = u[:].rearrange("p (k s) -> p k s", k=2)
                vv = v[:].rearrange("p (k s) -> p k s", k=2)
                nc.vector.tensor_tensor(
                    out=uv, in0=tv[:, :, 0 : Q // 2], in1=tv[:, :, Q // 2 : Q],
                    op=Alu.max,
                )
                nc.vector.tensor_tensor(
                    out=vv, in0=uv[:, :, 0 : Q // 4], in1=uv[:, :, Q // 4 : Q // 2],
                    op=Alu.max,
                )
                nc.vector.tensor_reduce(
                    out=red[:], in_=vv, axis=mybir.AxisListType.X, op=Alu.max
                )
                idx = pipe.intermediate_tile([P, 2], i32, bufs=rb, name="idx")
                nc.scalar.activation(
                    out=idx[:, 1:2], in_=red[:, 0:1], func=Act.Relu,
                    scale=64.0, bias=consts_sb[:, 0:1],
                )
                nc.scalar.activation(
                    out=idx[:, 0:1], in_=red[:, 1:2], func=Act.Relu,
                    scale=64.0, bias=consts_sb[:, 1:2],
                )
                return idx

            def gather(pipe, iv, idx):
                out_sb = pipe.intermediate_tile(
                    [P, 2 * H], f16, bufs=rb, name="out_sb"
                )
                nc.gpsimd.indirect_dma_start(
                    out=out_sb[:, H : 2 * H],
                    out_offset=None,
                    in_=inpf[:],
                    in_offset=IndirectOffsetOnAxis(ap=idx[:, 1:2], axis=0),
                    bounds_check=TOTR - 1,
                    oob_is_err=False,
                )
                nc.gpsimd.indirect_dma_start(
                    out=out_sb[:, 0:H],
                    out_offset=None,
                    in_=inpb[:],
                    in_offset=IndirectOffsetOnAxis(ap=idx[:, 0:1], axis=0),
                    bounds_check=TOTR - 1,
                    oob_is_err=False,
                )
                return out_sb

            def store(pipe, iv, out_sb):
                nc.sync.dma_start(out[:], out_sb[:])

            tc.For_i_pipelined(
                [load, mm, fold, gather, store],
                0,
                n_iters,
                unroll=unroll,
                staggered_reset=loop_iters is not None,
            )

    nc.compile()
    return nc


_NC_CACHE: bacc.Bacc | None = None


def _get_nc() -> bacc.Bacc:
    global _NC_CACHE
    if _NC_CACHE is None:
        _NC_CACHE = build_nc()
    return _NC_CACHE


def _weights() -> tuple[np.ndarray, np.ndarray]:
    l = np.tile(np.arange(1, J + 1, dtype=np.float64), BPC)
    b = np.arange(P) // J
    wa = np.zeros((K, P), np.float64)
    wa[0, :] = -CQ * l * l
    wa[3 + 2 * b, np.arange(P)] = -CQ
    wa[4 + 2 * b, np.arange(P)] = 2 * CQ * l
    wb = wa.copy()
    wa[1, :] = CI
    wb[2, :] = CI
    return wa.astype(np.float16), wb.astype(np.float16)


def make_in_maps(input: np.ndarray, number_mask: np.ndarray) -> list[dict]:
    b = np.arange(P, dtype=np.float32) // J
    consts_np = np.stack(
        [S * b, float((BPC - 1) * S) - S * b], axis=1
    ).astype(np.float32)
    wa_np, wb_np = _weights()
    iota_f = np.arange(1, S + 1, dtype=np.float64)
    statr_np = np.stack(
        [np.ones(S, np.float64), iota_f, (S + 1) - iota_f]
    ).astype(np.float16)
    mask_f = np.asarray(number_mask).astype(np.float16)
    inp_f16 = np.asarray(input).astype(np.float16)
    zrow = np.zeros((1, H), np.float16)
    in_maps = []
    for c in range(NCORES):
        sl = slice(c * BPC, (c + 1) * BPC)
        rows = inp_f16[sl].reshape(ROWS, H)
        mq = np.empty((2 * BPC, S), np.float16)
        m = mask_f[sl]
        mq[0::2] = (m.astype(np.float32) ** 2).astype(np.float16)
        mq[1::2] = m
        in_maps.append(
            {
                "inpf": np.concatenate([zrow, rows]),
                "inpb": np.concatenate([zrow, rows[::-1]]),
                "maskq": np.ascontiguousarray(mq),
                "statr": statr_np,
                "wa": wa_np,
                "wb": wb_np,
                "consts": consts_np,
            }
        )
    return in_maps


def kernel(input: np.ndarray, number_mask: np.ndarray, max_number=20) -> np.ndarray:
    assert int(max_number) == J
    nc = _get_nc()
    in_maps = make_in_maps(input, number_mask)
    res = run_bass_kernel_spmd(nc, in_maps, core_ids=list(range(NCORES)))
    outs = [
        res.results[c]["out"].reshape(BPC, J, 2 * H).astype(np.float32)
        for c in range(NCORES)
    ]
    return np.concatenate(outs, axis=0)


# revision 3
# speedup vs baseline: 1.0886x; 1.0886x over previous
"""v4 = v3 (quadratic-matmul t12, Relu-clamped zero-row gathers) with the
PSUM recycle cycle broken: each direction's PSUM is two independent
half-tiles (psA_L/psA_H, psB_L/psB_H, 2 banks each), so PE's refill of
one half overlaps ACT/DVE draining the other. The max-fold pairs
positions (s, s+512) within each half; an extra combine level merges
the halves. See kernel_v3.py for the full algebra/correctness argument
(pairing choice does not affect a max-reduction).
"""

import contextlib

import numpy as np

import concourse.bass as bass
import concourse.tile as tile
from concourse import bacc, mybir
from concourse.bass import IndirectOffsetOnAxis
from concourse.bass_utils import run_bass_kernel_spmd

B, S, H, J = 32, 2048, 1024, 20
NCORES = 8
BPC = B // NCORES
P = BPC * J
ROWS = BPC * S
TOTR = ROWS + 1
K = 2 * BPC + 3
CQ = 160.0
CI = 1.0 / 64.0
UNROLL = 32
MMCHUNK = 512
HS = S // 2   # 1024: columns per psum half-tile
Q = S // 4    # 512

f16 = mybir.dt.float16
f32 = mybir.dt.float32
i32 = mybir.dt.int32
Alu = mybir.AluOpType
Act = mybir.ActivationFunctionType


def build_nc(loop_iters: int | None = None) -> bacc.Bacc:
    nc = bacc.Bacc(
        "TRN2", target_bir_lowering=False, debug=False, num_devices=NCORES
    )
    inpf = nc.dram_tensor("inpf", [TOTR, H], f16, kind="ExternalInput").ap()
    inpb = nc.dram_tensor("inpb", [TOTR, H], f16, kind="ExternalInput").ap()
    maskq = nc.dram_tensor("maskq", [2 * BPC, S], f16, kind="ExternalInput").ap()
    statr = nc.dram_tensor("statr", [3, S], f16, kind="ExternalInput").ap()
    wa = nc.dram_tensor("wa", [K, P], f16, kind="ExternalInput").ap()
    wb = nc.dram_tensor("wb", [K, P], f16, kind="ExternalInput").ap()
    consts = nc.dram_tensor("consts", [P, 2], f32, kind="ExternalInput").ap()
    out = nc.dram_tensor("out", [P, 2 * H], f16, kind="ExternalOutput").ap()

    n_iters = loop_iters if loop_iters is not None else 1
    unroll = UNROLL if loop_iters is not None else 1
    bb = 2 if loop_iters is not None else 1
    rb = 8 if loop_iters is not None else 1

    with tile.TileContext(nc) as tc:
        with contextlib.ExitStack() as stk:
            static = stk.enter_context(tc.tile_pool(name="static", bufs=1))
            psum = stk.enter_context(
                tc.tile_pool(name="psum", bufs=1, space="PSUM")
            )
            consts_sb = static.tile([P, 2], f32)
            wa_sb = static.tile([K, P], f16)
            wb_sb = static.tile([K, P], f16)
            nc.scalar.dma_start(consts_sb[:], consts[:])
            nc.scalar.dma_start(wa_sb[:], wa[:])
            nc.scalar.dma_start(wb_sb[:], wb[:])

            maskring = [
                static.tile([K, S], f16, name=f"mq{i}") for i in range(rb)
            ]
            for t in maskring:
                nc.scalar.dma_start(t[0:3, :], statr[:])
            psAL = psum.tile([P, HS], f32, name="psAL")
            psAH = psum.tile([P, HS], f32, name="psAH")
            psBL = psum.tile([P, HS], f32, name="psBL")
            psBH = psum.tile([P, HS], f32, name="psBH")

            def load(pipe, iv):
                mq = pipe.intermediate_tile(
                    [K, S], f16, name="mq", bufs=rb, prealloc=maskring
                )
                nc.sync.dma_start(mq[3:11, :], maskq[:])
                return mq

            def mm(pipe, iv, mq):
                pA_L = pipe.intermediate_tile(
                    [P, HS], f32, bufs=1, name="pA_L", prealloc=[psAL]
                )
                pA_H = pipe.intermediate_tile(
                    [P, HS], f32, bufs=1, name="pA_H", prealloc=[psAH]
                )
                pB_L = pipe.intermediate_tile(
                    [P, HS], f32, bufs=1, name="pB_L", prealloc=[psBL]
                )
                pB_H = pipe.intermediate_tile(
                    [P, HS], f32, bufs=1, name="pB_H", prealloc=[psBH]
                )
                for dst, lo in ((pA_L, 0), (pA_H, HS)):
                    for n in range(0, HS, MMCHUNK):
                        nc.tensor.matmul(
                            dst[:, n : n + MMCHUNK], wa_sb[:],
                            mq[:, lo + n : lo + n + MMCHUNK],
                        )
                for dst, lo in ((pB_L, 0), (pB_H, HS)):
                    for n in range(0, HS, MMCHUNK):
                        nc.tensor.matmul(
                            dst[:, n : n + MMCHUNK], wb_sb[:],
                            mq[:, lo + n : lo + n + MMCHUNK],
                        )
                return pA_L, pA_H, pB_L, pB_H

            def fold(pipe, iv, ps):
                pA_L, pA_H, pB_L, pB_H = ps
                # ACT drains the low quarter of each A-half and all of B to
                # f16 (DVE may read at most one PSUM operand per op).
                aL = pipe.intermediate_tile([P, Q], f16, bufs=bb, name="aL")
                aH = pipe.intermediate_tile([P, Q], f16, bufs=bb, name="aH")
                b16 = pipe.intermediate_tile([P, S], f16, bufs=bb, name="b16")
                nc.scalar.activation(
                    out=aL[:], in_=pA_L[:, 0:Q], func=Act.Identity, scale=1.0
                )
                nc.scalar.activation(
                    out=b16[:, 0:HS], in_=pB_L[:], func=Act.Identity, scale=1.0
                )
                nc.scalar.activation(
                    out=aH[:], in_=pA_H[:, 0:Q], func=Act.Identity, scale=1.0
                )
                nc.scalar.activation(
                    out=b16[:, HS:S], in_=pB_H[:], func=Act.Identity, scale=1.0
                )
                # DVE folds: A halves pair (s, s+512) in-half vs f16 copy.
                fa = pipe.intermediate_tile([P, S // 2], f16, bufs=1, name="fa")
                nc.vector.tensor_tensor(
                    out=fa[:, 0:Q], in0=aL[:], in1=pA_L[:, Q:HS], op=Alu.max
                )
                nc.vector.tensor_tensor(
                    out=fa[:, Q : 2 * Q], in0=aH[:], in1=pA_H[:, Q:HS],
                    op=Alu.max,
                )
                # t512: dir A combined to 512, dir B folded 2048->512.
                t512 = pipe.intermediate_tile([P, 2 * Q], f16, bufs=1, name="t512")
                fb = pipe.intermediate_tile([P, S // 2], f16, bufs=1, name="fb")
                nc.vector.tensor_tensor(
                    out=t512[:, 0:Q], in0=fa[:, 0:Q], in1=fa[:, Q : 2 * Q],
                    op=Alu.max,
                )
                nc.vector.tensor_tensor(
                    out=fb[:], in0=b16[:, 0:HS], in1=b16[:, HS:S], op=Alu.max
                )
                nc.vector.tensor_tensor(
                    out=t512[:, Q : 2 * Q], in0=fb[:, 0:Q], in1=fb[:, Q : 2 * Q],
                    op=Alu.max,
                )
                # joint tree on [P, 2, 512]
                u = pipe.intermediate_tile([P, Q], f16, bufs=1, name="u")
                v = pipe.intermediate_tile([P, Q // 2], f16, bufs=1, name="v")
                red = pipe.intermediate_tile([P, 2], f16, bufs=rb, name="red")
                tv = t512[:].rearrange("p (k s) -> p k s", k=2)
                uv = u[:].rearrange("p (k s) -> p k s", k=2)
                vv = v[:].rearrange("p (k s) -> p k s", k=2)
                nc.vector.tensor_tensor(
                    out=uv, in0=tv[:, :, 0 : Q // 2], in1=tv[:, :, Q // 2 : Q],
                    op=Alu.max,
                )
                nc.vector.tensor_tensor(
                    out=vv, in0=uv[:, :, 0 : Q // 4], in1=uv[:, :, Q // 4 : Q // 2],
                    op=Alu.max,
                )
                nc.vector.tensor_reduce(
                    out=red[:], in_=vv, axis=mybir.AxisListType.X, op=Alu.max
                )
                idx = pipe.intermediate_tile([P, 2], i32, bufs=rb, name="idx")
                nc.scalar.activation(
                    out=idx[:, 1:2], in_=red[:, 0:1], func=Act.Relu,
                    scale=64.0, bias=consts_sb[:, 0:1],
                )
                nc.scalar.activation(
                    out=idx[:, 0:1], in_=red[:, 1:2], func=Act.Relu,
                    scale=64.0, bias=consts_sb[:, 1:2],
                )
                return idx

            def gather(pipe, iv, idx):
                out_sb = pipe.intermediate_tile(
                    [P, 2 * H], f16, bufs=rb, name="out_sb"
                )
                nc.gpsimd.indirect_dma_start(
                    out=out_sb[:, H : 2 * H],
                    out_offset=None,
                    in_=inpf[:],
                    in_offset=IndirectOffsetOnAxis(ap=idx[:, 1:2], axis=0),
                    bounds_check=TOTR - 1,
                    oob_is_err=False,
                )
                nc.gpsimd.indirect_dma_start(
                    out=out_sb[:, 0:H],
                    out_offset=None,
                    in_=inpb[:],
                    in_offset=IndirectOffsetOnAxis(ap=idx[:, 0:1], axis=0),
                    bounds_check=TOTR - 1,
                    oob_is_err=False,
                )
                return out_sb

            def store(pipe, iv, out_sb):
                nc.sync.dma_start(out[:], out_sb[:])

            tc.For_i_pipelined(
                [load, mm, fold, gather, store],
                0,
                n_iters,
                unroll=unroll,
                staggered_reset=loop_iters is not None,
            )

    nc.compile()
    return nc


_NC_CACHE: bacc.Bacc | None = None


def _get_nc() -> bacc.Bacc:
    global _NC_CACHE
    if _NC_CACHE is None:
        _NC_CACHE = build_nc()
    return _NC_CACHE


def _weights() -> tuple[np.ndarray, np.ndarray]:
    l = np.tile(np.arange(1, J + 1, dtype=np.float64), BPC)
    b = np.arange(P) // J
    wa = np.zeros((K, P), np.float64)
    wa[0, :] = -CQ * l * l
    wa[3 + 2 * b, np.arange(P)] = -CQ
    wa[4 + 2 * b, np.arange(P)] = 2 * CQ * l
    wb = wa.copy()
    wa[1, :] = CI
    wb[2, :] = CI
    return wa.astype(np.float16), wb.astype(np.float16)


def make_in_maps(input: np.ndarray, number_mask: np.ndarray) -> list[dict]:
    b = np.arange(P, dtype=np.float32) // J
    consts_np = np.stack(
        [S * b, float((BPC - 1) * S) - S * b], axis=1
    ).astype(np.float32)
    wa_np, wb_np = _weights()
    iota_f = np.arange(1, S + 1, dtype=np.float64)
    statr_np = np.stack(
        [np.ones(S, np.float64), iota_f, (S + 1) - iota_f]
    ).astype(np.float16)
    mask_f = np.asarray(number_mask).astype(np.float16)
    inp_f16 = np.asarray(input).astype(np.float16)
    zrow = np.zeros((1, H), np.float16)
    in_maps = []
    for c in range(NCORES):
        sl = slice(c * BPC, (c + 1) * BPC)
        rows = inp_f16[sl].reshape(ROWS, H)
        mq = np.empty((2 * BPC, S), np.float16)
        m = mask_f[sl]
        mq[0::2] = (m.astype(np.float32) ** 2).astype(np.float16)
        mq[1::2] = m
        in_maps.append(
            {
                "inpf": np.concatenate([zrow, rows]),
                "inpb": np.concatenate([zrow, rows[::-1]]),
                "maskq": np.ascontiguousarray(mq),
                "statr": statr_np,
                "wa": wa_np,
                "wb": wb_np,
                "consts": consts_np,
            }
        )
    return in_maps


def kernel(input: np.ndarray, number_mask: np.ndarray, max_number=20) -> np.ndarray:
    assert int(max_number) == J
    nc = _get_nc()
    in_maps = make_in_maps(input, number_mask)
    res = run_bass_kernel_spmd(nc, in_maps, core_ids=list(range(NCORES)))
    outs = [
        res.results[c]["out"].reshape(BPC, J, 2 * H).astype(np.float32)
        for c in range(NCORES)
    ]
    return np.concatenate(outs, axis=0)


# revision 4
# speedup vs baseline: 1.1220x; 1.0307x over previous
"""v4 = v3 (quadratic-matmul t12, Relu-clamped zero-row gathers) with the
PSUM recycle cycle broken: each direction's PSUM is two independent
half-tiles (psA_L/psA_H, psB_L/psB_H, 2 banks each), so PE's refill of
one half overlaps ACT/DVE draining the other. The max-fold pairs
positions (s, s+512) within each half; an extra combine level merges
the halves. See kernel_v3.py for the full algebra/correctness argument
(pairing choice does not affect a max-reduction).
"""

import contextlib

import numpy as np

import concourse.bass as bass
import concourse.tile as tile
from concourse import bacc, mybir
from concourse.bass import IndirectOffsetOnAxis
from concourse.bass_utils import run_bass_kernel_spmd

B, S, H, J = 32, 2048, 1024, 20
NCORES = 8
BPC = B // NCORES
P = BPC * J
ROWS = BPC * S
TOTR = ROWS + 1
K = 2 * BPC + 3
CQ = 160.0
CI = 1.0 / 64.0
UNROLL = 128
MMCHUNK = 512
HS = S // 2   # 1024: columns per psum half-tile
Q = S // 4    # 512

f16 = mybir.dt.float16
f32 = mybir.dt.float32
i32 = mybir.dt.int32
Alu = mybir.AluOpType
Act = mybir.ActivationFunctionType


def build_nc(loop_iters: int | None = None) -> bacc.Bacc:
    nc = bacc.Bacc(
        "TRN2", target_bir_lowering=False, debug=False, num_devices=NCORES
    )
    inpf = nc.dram_tensor("inpf", [TOTR, H], f16, kind="ExternalInput").ap()
    inpb = nc.dram_tensor("inpb", [TOTR, H], f16, kind="ExternalInput").ap()
    maskq = nc.dram_tensor("maskq", [2 * BPC, S], f16, kind="ExternalInput").ap()
    statr = nc.dram_tensor("statr", [3, S], f16, kind="ExternalInput").ap()
    wa = nc.dram_tensor("wa", [K, P], f16, kind="ExternalInput").ap()
    wb = nc.dram_tensor("wb", [K, P], f16, kind="ExternalInput").ap()
    consts = nc.dram_tensor("consts", [P, 2], f32, kind="ExternalInput").ap()
    out = nc.dram_tensor("out", [P, 2 * H], f16, kind="ExternalOutput").ap()

    n_iters = loop_iters if loop_iters is not None else 1
    unroll = UNROLL if loop_iters is not None else 1
    bb = 2 if loop_iters is not None else 1
    rb = 8 if loop_iters is not None else 1

    with tile.TileContext(nc) as tc:
        with contextlib.ExitStack() as stk:
            static = stk.enter_context(tc.tile_pool(name="static", bufs=1))
            psum = stk.enter_context(
                tc.tile_pool(name="psum", bufs=1, space="PSUM")
            )
            consts_sb = static.tile([P, 2], f32)
            wa_sb = static.tile([K, P], f16)
            wb_sb = static.tile([K, P], f16)
            nc.scalar.dma_start(consts_sb[:], consts[:])
            nc.scalar.dma_start(wa_sb[:], wa[:])
            nc.scalar.dma_start(wb_sb[:], wb[:])

            maskring = [
                static.tile([K, S], f16, name=f"mq{i}") for i in range(rb)
            ]
            for t in maskring:
                nc.scalar.dma_start(t[0:3, :], statr[:])
            psAL = psum.tile([P, HS], f32, name="psAL")
            psAH = psum.tile([P, HS], f32, name="psAH")
            psBL = psum.tile([P, HS], f32, name="psBL")
            psBH = psum.tile([P, HS], f32, name="psBH")

            def load(pipe, iv):
                mq = pipe.intermediate_tile(
                    [K, S], f16, name="mq", bufs=rb, prealloc=maskring
                )
                nc.sync.dma_start(mq[3:11, :], maskq[:])
                return mq

            def mm(pipe, iv, mq):
                pA_L = pipe.intermediate_tile(
                    [P, HS], f32, bufs=1, name="pA_L", prealloc=[psAL]
                )
                pA_H = pipe.intermediate_tile(
                    [P, HS], f32, bufs=1, name="pA_H", prealloc=[psAH]
                )
                pB_L = pipe.intermediate_tile(
                    [P, HS], f32, bufs=1, name="pB_L", prealloc=[psBL]
                )
                pB_H = pipe.intermediate_tile(
                    [P, HS], f32, bufs=1, name="pB_H", prealloc=[psBH]
                )
                for dst, lo in ((pA_L, 0), (pA_H, HS)):
                    for n in range(0, HS, MMCHUNK):
                        nc.tensor.matmul(
                            dst[:, n : n + MMCHUNK], wa_sb[:],
                            mq[:, lo + n : lo + n + MMCHUNK],
                        )
                for dst, lo in ((pB_L, 0), (pB_H, HS)):
                    for n in range(0, HS, MMCHUNK):
                        nc.tensor.matmul(
                            dst[:, n : n + MMCHUNK], wb_sb[:],
                            mq[:, lo + n : lo + n + MMCHUNK],
                        )
                return pA_L, pA_H, pB_L, pB_H

            def fold(pipe, iv, ps):
                pA_L, pA_H, pB_L, pB_H = ps
                # ACT drains the low quarter of each A-half and all of B to
                # f16 (DVE may read at most one PSUM operand per op).
                aL = pipe.intermediate_tile([P, Q], f16, bufs=bb, name="aL")
                aH = pipe.intermediate_tile([P, Q], f16, bufs=bb, name="aH")
                b16 = pipe.intermediate_tile([P, S], f16, bufs=bb, name="b16")
                nc.scalar.activation(
                    out=aL[:], in_=pA_L[:, 0:Q], func=Act.Identity, scale=1.0
                )
                nc.scalar.activation(
                    out=b16[:, 0:HS], in_=pB_L[:], func=Act.Identity, scale=1.0
                )
                nc.scalar.activation(
                    out=aH[:], in_=pA_H[:, 0:Q], func=Act.Identity, scale=1.0
                )
                nc.scalar.activation(
                    out=b16[:, HS:S], in_=pB_H[:], func=Act.Identity, scale=1.0
                )
                # DVE folds: A halves pair (s, s+512) in-half vs f16 copy.
                fa = pipe.intermediate_tile([P, S // 2], f16, bufs=1, name="fa")
                nc.vector.tensor_tensor(
                    out=fa[:, 0:Q], in0=aL[:], in1=pA_L[:, Q:HS], op=Alu.max
                )
                nc.vector.tensor_tensor(
                    out=fa[:, Q : 2 * Q], in0=aH[:], in1=pA_H[:, Q:HS],
                    op=Alu.max,
                )
                # t512: dir A combined to 512, dir B folded 2048->512.
                t512 = pipe.intermediate_tile([P, 2 * Q], f16, bufs=1, name="t512")
                fb = pipe.intermediate_tile([P, S // 2], f16, bufs=1, name="fb")
                nc.vector.tensor_tensor(
                    out=t512[:, 0:Q], in0=fa[:, 0:Q], in1=fa[:, Q : 2 * Q],
                    op=Alu.max,
                )
                nc.vector.tensor_tensor(
                    out=fb[:], in0=b16[:, 0:HS], in1=b16[:, HS:S], op=Alu.max
                )
                nc.vector.tensor_tensor(
                    out=t512[:, Q : 2 * Q], in0=fb[:, 0:Q], in1=fb[:, Q : 2 * Q],
                    op=Alu.max,
                )
                # joint tree on [P, 2, 512]
                u = pipe.intermediate_tile([P, Q], f16, bufs=1, name="u")
                v = pipe.intermediate_tile([P, Q // 2], f16, bufs=1, name="v")
                red = pipe.intermediate_tile([P, 2], f16, bufs=rb, name="red")
                tv = t512[:].rearrange("p (k s) -> p k s", k=2)
                uv = u[:].rearrange("p (k s) -> p k s", k=2)
                vv = v[:].rearrange("p (k s) -> p k s", k=2)
                nc.vector.tensor_tensor(
                    out=uv, in0=tv[:, :, 0 : Q // 2], in1=tv[:, :, Q // 2 : Q],
                    op=Alu.max,
                )
                nc.vector.tensor_tensor(
                    out=vv, in0=uv[:, :, 0 : Q // 4], in1=uv[:, :, Q // 4 : Q // 2],
                    op=Alu.max,
                )
                nc.vector.tensor_reduce(
                    out=red[:], in_=vv, axis=mybir.AxisListType.X, op=Alu.max
                )
                idx = pipe.intermediate_tile([P, 2], i32, bufs=rb, name="idx")
                nc.scalar.activation(
                    out=idx[:, 1:2], in_=red[:, 0:1], func=Act.Relu,
                    scale=64.0, bias=consts_sb[:, 0:1],
                )
                nc.scalar.activation(
                    out=idx[:, 0:1], in_=red[:, 1:2], func=Act.Relu,
                    scale=64.0, bias=consts_sb[:, 1:2],
                )
                return idx

            def gather(pipe, iv, idx):
                out_sb = pipe.intermediate_tile(
                    [P, 2 * H], f16, bufs=rb, name="out_sb"
                )
                nc.gpsimd.indirect_dma_start(
                    out=out_sb[:, H : 2 * H],
                    out_offset=None,
                    in_=inpf[:],
                    in_offset=IndirectOffsetOnAxis(ap=idx[:, 1:2], axis=0),
                    bounds_check=TOTR - 1,
                    oob_is_err=False,
                )
                nc.gpsimd.indirect_dma_start(
                    out=out_sb[:, 0:H],
                    out_offset=None,
                    in_=inpb[:],
                    in_offset=IndirectOffsetOnAxis(ap=idx[:, 0:1], axis=0),
                    bounds_check=TOTR - 1,
                    oob_is_err=False,
                )
                return out_sb

            def store(pipe, iv, out_sb):
                nc.sync.dma_start(out[:], out_sb[:])

            tc.For_i_pipelined(
                [load, mm, fold, gather, store],
                0,
                n_iters,
                unroll=unroll,
                staggered_reset=loop_iters is not None,
            )

    nc.compile()
    return nc


_NC_CACHE: bacc.Bacc | None = None


def _get_nc() -> bacc.Bacc:
    global _NC_CACHE
    if _NC_CACHE is None:
        _NC_CACHE = build_nc()
    return _NC_CACHE


def _weights() -> tuple[np.ndarray, np.ndarray]:
    l = np.tile(np.arange(1, J + 1, dtype=np.float64), BPC)
    b = np.arange(P) // J
    wa = np.zeros((K, P), np.float64)
    wa[0, :] = -CQ * l * l
    wa[3 + 2 * b, np.arange(P)] = -CQ
    wa[4 + 2 * b, np.arange(P)] = 2 * CQ * l
    wb = wa.copy()
    wa[1, :] = CI
    wb[2, :] = CI
    return wa.astype(np.float16), wb.astype(np.float16)


def make_in_maps(input: np.ndarray, number_mask: np.ndarray) -> list[dict]:
    b = np.arange(P, dtype=np.float32) // J
    consts_np = np.stack(
        [S * b, float((BPC - 1) * S) - S * b], axis=1
    ).astype(np.float32)
    wa_np, wb_np = _weights()
    iota_f = np.arange(1, S + 1, dtype=np.float64)
    statr_np = np.stack(
        [np.ones(S, np.float64), iota_f, (S + 1) - iota_f]
    ).astype(np.float16)
    mask_f = np.asarray(number_mask).astype(np.float16)
    inp_f16 = np.asarray(input).astype(np.float16)
    zrow = np.zeros((1, H), np.float16)
    in_maps = []
    for c in range(NCORES):
        sl = slice(c * BPC, (c + 1) * BPC)
        rows = inp_f16[sl].reshape(ROWS, H)
        mq = np.empty((2 * BPC, S), np.float16)
        m = mask_f[sl]
        mq[0::2] = (m.astype(np.float32) ** 2).astype(np.float16)
        mq[1::2] = m
        in_maps.append(
            {
                "inpf": np.concatenate([zrow, rows]),
                "inpb": np.concatenate([zrow, rows[::-1]]),
                "maskq": np.ascontiguousarray(mq),
                "statr": statr_np,
                "wa": wa_np,
                "wb": wb_np,
                "consts": consts_np,
            }
        )
    return in_maps


def kernel(input: np.ndarray, number_mask: np.ndarray, max_number=20) -> np.ndarray:
    assert int(max_number) == J
    nc = _get_nc()
    in_maps = make_in_maps(input, number_mask)
    res = run_bass_kernel_spmd(nc, in_maps, core_ids=list(range(NCORES)))
    outs = [
        res.results[c]["out"].reshape(BPC, J, 2 * H).astype(np.float32)
        for c in range(NCORES)
    ]
    return np.concatenate(outs, axis=0)
